# revision 4
# baseline (speedup 1.0000x reference)
"""Trainium2 Bass kernel for KPToSkl: keypoint-skeleton Gaussian heatmap.

Computes heat[b,y,x] = max_e exp(-gamma * dist^2(pixel, segment_e))
                     = exp(-gamma * min_e dist^2)   (exp is monotone)

Data-parallel over batch: B=32 split as 4 batches per NeuronCore x 8 cores.

Per (b, e) the device computes, over the 256x256 image laid out as
[128 partitions (y mod 128), 512 free (yhalf*256 + x)]:

  PE:  A  = projection coordinate, affine in (y, x), scaled so the segment
            band maps to [-1, 1]  (rank-k matmul vs a shared x-basis)
       W2 = gamma * perpendicular_dist^2, quadratic in (y, x) (rank-k matmul)
  DVE: m  = max(|A|, 1) - 1          (tensor_scalar dual-op: abs_max, subtract)
  ACT: q  = (sigma*m)^2              (Square activation, per-(b,e) scale col)
  DVE: d2 = W2 + q ; M = min(M, d2)
  ACT: heat = exp(-M) at the end of the edge loop.

All per-(b,e) coefficients are host-precomputed into fp16 hi/lo pairs
(products of fp16 values accumulate exactly in fp32 PSUM), so the matmuls
run at the full 1 cycle/row fp16 rate with ~1e-6 accuracy.
"""

from contextlib import ExitStack

import numpy as np

import concourse.bass as bass
import concourse.tile as tile
from concourse import bacc, mybir
from concourse.bass_utils import run_bass_kernel_spmd

N_CORES = 8
B_TOTAL = 32
B_LOCAL = B_TOTAL // N_CORES  # 4
E = 18
H = W = 256
GAMMA = 0.2
SG = float(np.sqrt(GAMMA))
BE = B_LOCAL * E  # 72 (b,e) pairs per core

KA = 6   # lhsT rows for the A matmul
KW = 11  # lhsT rows for the W2 matmul

F16 = mybir.dt.float16
F32 = mybir.dt.float32

_cache = {}


def _split16(v):
    v = np.asarray(v, np.float64)
    hi = v.astype(np.float16)
    lo = (v - hi.astype(np.float64)).astype(np.float16)
    return hi, lo


def _basis_tables():
    """Shared rhs basis matrices (same for every core / batch / edge).

    The x basis is scaled by 2^-8 (exact in fp16) so that the per-edge
    coefficients stay in fp16's *normal* range — subnormal coefficients
    (< 6.1e-5) have an absolute quantization floor of ~6e-8 that, times
    x^2 ~ 65025, would cost ~2e-3 of absolute error.
    """
    x = np.arange(W, dtype=np.float64)
    xs = x / 256.0  # exact in fp16 (power-of-2 scale of integers < 2^11)
    ones = np.ones(W, np.float64)
    zero = np.zeros(W, np.float64)
    xs2 = xs * xs
    xs2h, xs2l = _split16(xs2)
    xs2h = xs2h.astype(np.float64)
    xs2l = xs2l.astype(np.float64)

    def row(a, b):
        return np.concatenate([a, b])

    rhsA = np.stack([
        row(xs, xs),      # c1_hi
        row(xs, xs),      # c1_lo
        row(ones, zero),  # d_hi yt0
        row(ones, zero),  # d_lo yt0
        row(zero, ones),  # d_hi yt1
        row(zero, ones),  # d_lo yt1
    ])
    rhsW = np.stack([
        row(xs2h, xs2h),  # C_hi
        row(xs2l, xs2l),  # C_hi
        row(xs2h, xs2h),  # C_lo
        row(xs, zero),    # f_hi yt0
        row(xs, zero),    # f_lo yt0
        row(zero, xs),    # f_hi yt1
        row(zero, xs),    # f_lo yt1
        row(ones, zero),  # g_hi yt0
        row(ones, zero),  # g_lo yt0
        row(zero, ones),  # g_hi yt1
        row(zero, ones),  # g_lo yt1
    ])
    return rhsA.astype(np.float16), rhsW.astype(np.float16)


def _core_tables(kps_core):
    """Per-core lhsT coefficient tables + sigma columns.

    kps_core: [B_LOCAL, 18, 2] float (x, y) keypoints in [-1, 1].
    Returns lhsA [KA, BE*128] f16, lhsW [KW, BE*128] f16, sig [128, BE] f32.
    """
    ky = kps_core[:, :, 1].astype(np.float64)
    kx = kps_core[:, :, 0].astype(np.float64)
    PI = np.arange(E)
    PJ = (np.arange(E) + 1) % E
    piy, pix = ky[:, PI], kx[:, PI]
    pjy, pjx = ky[:, PJ], kx[:, PJ]
    vy, vx = piy - pjy, pix - pjx
    vn = np.maximum(vy * vy + vx * vx, 1e-12)
    s = np.sqrt(vn)

    P = 2 * vy / vn
    Q = 2 * vx / vn
    R = -1 - (2 / vn) * (pjy * vy + pjx * vx)
    c1 = 2 * Q * 256.0 / 255.0  # coefficient of xs = x/256

    G = SG * vx / s
    Hc = -SG * vy / s
    J = SG * (pjx * vy - pjy * vx) / s
    c2 = 2 * Hc * 256.0 / 255.0  # coefficient of xs in w
    C = c2 * c2

    p = np.arange(128, dtype=np.float64)
    yc0 = 2 * p / 255.0 - 1
    yc1 = 2 * (p + 128) / 255.0 - 1

    lhsA = np.zeros((KA, BE, 128), np.float16)
    lhsW = np.zeros((KW, BE, 128), np.float16)
    sig = np.zeros((128, BE), np.float32)

    for b in range(B_LOCAL):
        for e in range(E):
            be = b * E + e
            c1h, c1l = _split16(c1[b, e])
            lhsA[0, be, :] = c1h
            lhsA[1, be, :] = c1l
            for yt, yc in ((0, yc0), (1, yc1)):
                d = P[b, e] * yc + (R[b, e] - Q[b, e])
                dh, dl = _split16(d)
                lhsA[2 + 2 * yt, be, :] = dh
                lhsA[3 + 2 * yt, be, :] = dl

            Ch, Cl = _split16(C[b, e])
            lhsW[0, be, :] = Ch
            lhsW[1, be, :] = Ch
            lhsW[2, be, :] = Cl
            for yt, yc in ((0, yc0), (1, yc1)):
                ecol = G[b, e] * yc + (J[b, e] - Hc[b, e])
                f = 2 * c2[b, e] * ecol
                g = ecol * ecol
                fh, fl = _split16(f)
                gh, gl = _split16(g)
                lhsW[3 + 2 * yt, be, :] = fh
                lhsW[4 + 2 * yt, be, :] = fl
                lhsW[7 + 2 * yt, be, :] = gh
                lhsW[8 + 2 * yt, be, :] = gl

            sig[:, be] = SG * s[b, e] / 2

    return (
        lhsA.reshape(KA, BE * 128),
        lhsW.reshape(KW, BE * 128),
        sig,
    )


def _build_program():
    nc = bacc.Bacc(
        "TRN2",
        target_bir_lowering=False,
        debug=False,
        num_devices=N_CORES,
    )

    lhsA_d = nc.declare_dram_parameter("lhsA", [KA, BE * 128], F16, isOutput=False)
    lhsW_d = nc.declare_dram_parameter("lhsW", [KW, BE * 128], F16, isOutput=False)
    rhsA_d = nc.declare_dram_parameter("rhsA", [KA, 512], F16, isOutput=False)
    rhsW_d = nc.declare_dram_parameter("rhsW", [KW, 512], F16, isOutput=False)
    sig_d = nc.declare_dram_parameter("sig", [128, BE], F32, isOutput=False)
    out_d = nc.declare_dram_parameter("out", [B_LOCAL, H, W], F32, isOutput=True)

    with tile.TileContext(nc) as tc, ExitStack() as ctx:
        const = ctx.enter_context(tc.tile_pool(name="const", bufs=1))
        psA = ctx.enter_context(tc.tile_pool(name="psA", bufs=3, space="PSUM"))
        psD = ctx.enter_context(tc.tile_pool(name="psD", bufs=3, space="PSUM"))
        work = ctx.enter_context(tc.tile_pool(name="work", bufs=4))
        mpool = ctx.enter_context(tc.tile_pool(name="mins", bufs=4))
        opool = ctx.enter_context(tc.tile_pool(name="outs", bufs=2))

        lhsA_t = const.tile([KA, BE * 128], F16)
        nc.sync.dma_start(lhsA_t[:], lhsA_d.ap())
        lhsW_t = const.tile([KW, BE * 128], F16)
        nc.sync.dma_start(lhsW_t[:], lhsW_d.ap())
        rhsA_t = const.tile([KA, 512], F16)
        nc.sync.dma_start(rhsA_t[:], rhsA_d.ap())
        rhsW_t = const.tile([KW, 512], F16)
        nc.sync.dma_start(rhsW_t[:], rhsW_d.ap())
        sig_t = const.tile([128, BE], F32)
        nc.sync.dma_start(sig_t[:], sig_d.ap())
        zcol = const.tile([128, 1], F32)
        nc.gpsimd.memset(zcol[:], 0.0)

        out_ap = out_d.ap()

        for b in range(B_LOCAL):
            # Ping-pong min accumulators (avoids in-place DVE read/write).
            m0 = mpool.tile([128, 512], F32, tag="m0")
            m1 = mpool.tile([128, 512], F32, tag="m1")
            nc.gpsimd.memset(m0[:], 30.0)
            cur, nxt = m0, m1
            for e in range(E):
                be = b * E + e
                pa = psA.tile([128, 512], F32)
                nc.tensor.matmul(
                    pa[:], lhsA_t[:, be * 128:(be + 1) * 128], rhsA_t[:],
                    start=True, stop=True,
                )
                pd = psD.tile([128, 512], F32)
                nc.tensor.matmul(
                    pd[:], lhsW_t[:, be * 128:(be + 1) * 128], rhsW_t[:],
                    start=True, stop=True,
                )
                zt = work.tile([128, 512], F32, tag="zt")
                nc.scalar.activation(
                    zt[:], pa[:], mybir.ActivationFunctionType.Abs,
                    bias=zcol[:], scale=1.0,
                )
                mt = work.tile([128, 512], F32, tag="mt")
                nc.vector.tensor_scalar(
                    mt[:], zt[:], 1.0, 1.0,
                    mybir.AluOpType.max, mybir.AluOpType.subtract,
                )
                qt = work.tile([128, 512], F32, tag="qt")
                nc.scalar.activation(
                    qt[:], mt[:], mybir.ActivationFunctionType.Square,
                    bias=zcol[:], scale=sig_t[:, be:be + 1],
                )
                dt = work.tile([128, 512], F32, tag="dt")
                nc.vector.tensor_tensor(dt[:], pd[:], qt[:], mybir.AluOpType.add)
                nc.vector.tensor_tensor(nxt[:], cur[:], dt[:], mybir.AluOpType.min)
                cur, nxt = nxt, cur
            ot = opool.tile([128, 512], F32)
            nc.scalar.activation(
                ot[:], cur[:], mybir.ActivationFunctionType.Exp,
                bias=zcol[:], scale=-1.0,
            )
            nc.sync.dma_start(out_ap[b, 0:128, :], ot[:, 0:256])
            nc.sync.dma_start(out_ap[b, 128:256, :], ot[:, 256:512])

    nc.compile()
    return nc


def _get_program():
    if "nc" not in _cache:
        _cache["nc"] = _build_program()
    return _cache["nc"]


def kernel(kps: np.ndarray) -> np.ndarray:
    kps = np.asarray(kps, np.float32)
    assert kps.shape == (B_TOTAL, E, 2), kps.shape

    nc = _get_program()
    rhsA, rhsW = _basis_tables()
    in_maps = []
    for c in range(N_CORES):
        lhsA, lhsW, sig = _core_tables(kps[c * B_LOCAL:(c + 1) * B_LOCAL])
        in_maps.append({
            "lhsA": lhsA, "lhsW": lhsW,
            "rhsA": rhsA, "rhsW": rhsW,
            "sig": sig,
        })

    res = run_bass_kernel_spmd(nc, in_maps, list(range(N_CORES)))
    out = np.concatenate([res.results[c]["out"] for c in range(N_CORES)], axis=0)
    return out.astype(np.float32)


# revision 9
# speedup vs baseline: 1.0745x; 1.0745x over previous
"""Trainium2 Bass kernel for KPToSkl: keypoint-skeleton Gaussian heatmap.

Computes heat[b,y,x] = max_e exp(-gamma * dist^2(pixel, segment_e))
                     = exp(-gamma * min_e dist^2)   (exp is monotone)

Data-parallel over batch: B=32 split as 4 batches per NeuronCore x 8 cores.

Per (b, e) the device computes, over the 256x256 image laid out as
[128 partitions (y mod 128), 512 free (yhalf*256 + x)]:

  PE:  A  = projection coordinate, affine in (y, x), scaled so the segment
            band maps to [-1, 1]  (rank-k matmul vs a shared x-basis)
       W2 = gamma * perpendicular_dist^2, quadratic in (y, x) (rank-k matmul)
  DVE: m  = max(|A|, 1) - 1          (tensor_scalar dual-op: abs_max, subtract)
  ACT: q  = (sigma*m)^2              (Square activation, per-(b,e) scale col)
  DVE: d2 = W2 + q ; M = min(M, d2)
  ACT: heat = exp(-M) at the end of the edge loop.

All per-(b,e) coefficients are host-precomputed into fp16 hi/lo pairs
(products of fp16 values accumulate exactly in fp32 PSUM), so the matmuls
run at the full 1 cycle/row fp16 rate with ~1e-6 accuracy.
"""

from contextlib import ExitStack

import numpy as np

import concourse.bass as bass
import concourse.tile as tile
from concourse import bacc, mybir
from concourse.bass_utils import run_bass_kernel_spmd

N_CORES = 8
B_TOTAL = 32
B_LOCAL = B_TOTAL // N_CORES  # 4
E = 18
H = W = 256
GAMMA = 0.2
SG = float(np.sqrt(GAMMA))
BE = B_LOCAL * E  # 72 (b,e) pairs per core

KA = 6   # lhsT rows for the A matmul
KW = 11  # lhsT rows for the W2 matmul

F16 = mybir.dt.float16
F32 = mybir.dt.float32

_cache = {}


def _split16(v):
    v = np.asarray(v, np.float64)
    hi = v.astype(np.float16)
    lo = (v - hi.astype(np.float64)).astype(np.float16)
    return hi, lo


def _basis_tables():
    """Shared rhs basis matrices (same for every core / batch / edge).

    The x basis is scaled by 2^-8 (exact in fp16) so that the per-edge
    coefficients stay in fp16's *normal* range — subnormal coefficients
    (< 6.1e-5) have an absolute quantization floor of ~6e-8 that, times
    x^2 ~ 65025, would cost ~2e-3 of absolute error.
    """
    x = np.arange(W, dtype=np.float64)
    xs = x / 256.0  # exact in fp16 (power-of-2 scale of integers < 2^11)
    ones = np.ones(W, np.float64)
    zero = np.zeros(W, np.float64)
    xs2 = xs * xs
    xs2h, xs2l = _split16(xs2)
    xs2h = xs2h.astype(np.float64)
    xs2l = xs2l.astype(np.float64)

    def row(a, b):
        return np.concatenate([a, b])

    rhsA = np.stack([
        row(xs, xs),      # c1_hi
        row(xs, xs),      # c1_lo
        row(ones, zero),  # d_hi yt0
        row(ones, zero),  # d_lo yt0
        row(zero, ones),  # d_hi yt1
        row(zero, ones),  # d_lo yt1
    ])
    rhsW = np.stack([
        row(xs2h, xs2h),  # C_hi
        row(xs2l, xs2l),  # C_hi
        row(xs2h, xs2h),  # C_lo
        row(xs, zero),    # f_hi yt0
        row(xs, zero),    # f_lo yt0
        row(zero, xs),    # f_hi yt1
        row(zero, xs),    # f_lo yt1
        row(ones, zero),  # g_hi yt0
        row(ones, zero),  # g_lo yt0
        row(zero, ones),  # g_hi yt1
        row(zero, ones),  # g_lo yt1
    ])
    return rhsA.astype(np.float16), rhsW.astype(np.float16)


def _core_tables(kps_core):
    """Per-core lhsT coefficient tables + sigma columns.

    kps_core: [B_LOCAL, 18, 2] float (x, y) keypoints in [-1, 1].
    Returns lhsA [KA, BE*128] f16, lhsW [KW, BE*128] f16, sig [128, BE] f32.
    """
    ky = kps_core[:, :, 1].astype(np.float64)
    kx = kps_core[:, :, 0].astype(np.float64)
    PI = np.arange(E)
    PJ = (np.arange(E) + 1) % E
    piy, pix = ky[:, PI], kx[:, PI]
    pjy, pjx = ky[:, PJ], kx[:, PJ]
    vy, vx = piy - pjy, pix - pjx
    vn = np.maximum(vy * vy + vx * vx, 1e-12)
    s = np.sqrt(vn)

    P = 2 * vy / vn
    Q = 2 * vx / vn
    R = -1 - (2 / vn) * (pjy * vy + pjx * vx)
    c1 = 2 * Q * 256.0 / 255.0  # coefficient of xs = x/256

    G = SG * vx / s
    Hc = -SG * vy / s
    J = SG * (pjx * vy - pjy * vx) / s
    c2 = 2 * Hc * 256.0 / 255.0  # coefficient of xs in w
    C = c2 * c2

    p = np.arange(128, dtype=np.float64)
    yc0 = 2 * p / 255.0 - 1
    yc1 = 2 * (p + 128) / 255.0 - 1

    lhsA = np.zeros((KA, BE, 128), np.float16)
    lhsW = np.zeros((KW, BE, 128), np.float16)
    sig = np.zeros((128, BE), np.float32)

    for b in range(B_LOCAL):
        for e in range(E):
            be = b * E + e
            c1h, c1l = _split16(c1[b, e])
            lhsA[0, be, :] = c1h
            lhsA[1, be, :] = c1l
            for yt, yc in ((0, yc0), (1, yc1)):
                d = P[b, e] * yc + (R[b, e] - Q[b, e])
                dh, dl = _split16(d)
                lhsA[2 + 2 * yt, be, :] = dh
                lhsA[3 + 2 * yt, be, :] = dl

            Ch, Cl = _split16(C[b, e])
            lhsW[0, be, :] = Ch
            lhsW[1, be, :] = Ch
            lhsW[2, be, :] = Cl
            for yt, yc in ((0, yc0), (1, yc1)):
                ecol = G[b, e] * yc + (J[b, e] - Hc[b, e])
                f = 2 * c2[b, e] * ecol
                g = ecol * ecol
                fh, fl = _split16(f)
                gh, gl = _split16(g)
                lhsW[3 + 2 * yt, be, :] = fh
                lhsW[4 + 2 * yt, be, :] = fl
                lhsW[7 + 2 * yt, be, :] = gh
                lhsW[8 + 2 * yt, be, :] = gl

            sig[:, be] = SG * s[b, e] / 2

    return (
        lhsA.reshape(KA, BE * 128),
        lhsW.reshape(KW, BE * 128),
        sig,
    )


DVE_Q_MOD = 5  # every DVE_Q_MOD-th edge computes its square on DVE (phi split)


def _build_program():
    nc = bacc.Bacc(
        "TRN2",
        target_bir_lowering=False,
        debug=False,
        num_devices=N_CORES,
    )

    lhsA_d = nc.declare_dram_parameter("lhsA", [KA, BE * 128], F16, isOutput=False)
    lhsW_d = nc.declare_dram_parameter("lhsW", [KW, BE * 128], F16, isOutput=False)
    rhsA_d = nc.declare_dram_parameter("rhsA", [KA, 512], F16, isOutput=False)
    rhsW_d = nc.declare_dram_parameter("rhsW", [KW, 512], F16, isOutput=False)
    sig_d = nc.declare_dram_parameter("sig", [128, BE], F32, isOutput=False)
    sig2_d = nc.declare_dram_parameter("sig2", [128, BE], F32, isOutput=False)
    ident_d = nc.declare_dram_parameter("ident", [128, 128], F16, isOutput=False)
    out_d = nc.declare_dram_parameter("out", [B_LOCAL, H, W], F32, isOutput=True)

    with tile.TileContext(nc) as tc, ExitStack() as ctx:
        const = ctx.enter_context(tc.tile_pool(name="const", bufs=1))
        psA = ctx.enter_context(tc.tile_pool(name="psA", bufs=2, space="PSUM"))
        psD = ctx.enter_context(tc.tile_pool(name="psD", bufs=4, space="PSUM"))
        work = ctx.enter_context(tc.tile_pool(name="work", bufs=3))
        qpool = ctx.enter_context(tc.tile_pool(name="qp", bufs=4))
        mpool = ctx.enter_context(tc.tile_pool(name="mins", bufs=4))
        opool = ctx.enter_context(tc.tile_pool(name="outs", bufs=2))

        lhsA_t = const.tile([KA, BE * 128], F16)
        nc.sync.dma_start(lhsA_t[:], lhsA_d.ap())
        lhsW_t = const.tile([KW, BE * 128], F16)
        nc.sync.dma_start(lhsW_t[:], lhsW_d.ap())
        rhsA_t = const.tile([KA, 512], F16)
        nc.sync.dma_start(rhsA_t[:], rhsA_d.ap())
        rhsW_t = const.tile([KW, 512], F16)
        nc.sync.dma_start(rhsW_t[:], rhsW_d.ap())
        sig_t = const.tile([128, BE], F32)
        nc.sync.dma_start(sig_t[:], sig_d.ap())
        sig2_t = const.tile([128, BE], F32)
        nc.sync.dma_start(sig2_t[:], sig2_d.ap())
        ident_t = const.tile([128, 128], F16)
        nc.sync.dma_start(ident_t[:], ident_d.ap())
        zcol = const.tile([128, 1], F32)
        nc.gpsimd.memset(zcol[:], 0.0)

        out_ap = out_d.ap()

        for b in range(B_LOCAL):
            # Ping-pong min accumulators (avoids in-place DVE read/write).
            m0 = mpool.tile([128, 512], F32, tag="m0")
            m1 = mpool.tile([128, 512], F32, tag="m1")
            nc.gpsimd.memset(m0[:], 30.0)
            cur, nxt = m0, m1
            for ep in range(E // 2):
                e0, e1 = 2 * ep, 2 * ep + 1
                be0, be1 = b * E + e0, b * E + e1

                aa = psA.tile([128, 1024], F32)
                pds = []
                for half, be in ((0, be0), (1, be1)):
                    nc.tensor.matmul(
                        aa[:, half * 512:(half + 1) * 512],
                        lhsA_t[:, be * 128:(be + 1) * 128], rhsA_t[:],
                        start=True, stop=True,
                    )
                    pd = psD.tile([128, 512], F32)
                    nc.tensor.matmul(
                        pd[:], lhsW_t[:, be * 128:(be + 1) * 128], rhsW_t[:],
                        start=True, stop=False,
                    )
                    pds.append(pd)

                zt = work.tile([128, 1024], F32, tag="zt")
                nc.scalar.activation(
                    zt[:], aa[:], mybir.ActivationFunctionType.Abs,
                    bias=zcol[:], scale=1.0,
                )
                mt = work.tile([128, 1024], F32, tag="mt")
                nc.vector.tensor_scalar(
                    mt[:], zt[:], 1.0, 1.0,
                    mybir.AluOpType.max, mybir.AluOpType.subtract,
                )
                for half, be in ((0, be0), (1, be1)):
                    mh = mt[:, half * 512:(half + 1) * 512]
                    qt = qpool.tile([128, 512], F16, tag="qt")
                    if (be % DVE_Q_MOD) == DVE_Q_MOD - 1:
                        # sigma^2 * m * m on DVE
                        nc.vector.scalar_tensor_tensor(
                            qt[:], mh, sig2_t[:, be:be + 1], mh,
                            mybir.AluOpType.mult, mybir.AluOpType.mult,
                        )
                    else:
                        nc.scalar.activation(
                            qt[:], mh, mybir.ActivationFunctionType.Square,
                            bias=zcol[:], scale=sig_t[:, be:be + 1],
                        )
                    pd = pds[half]
                    nc.tensor.matmul(
                        pd[:], ident_t[:], qt[:],
                        start=False, stop=True,
                    )
                    nc.vector.tensor_tensor(
                        nxt[:], cur[:], pd[:], mybir.AluOpType.min
                    )
                    cur, nxt = nxt, cur
            ot = opool.tile([128, 512], F32)
            nc.scalar.activation(
                ot[:], cur[:], mybir.ActivationFunctionType.Exp,
                bias=zcol[:], scale=-1.0,
            )
            nc.sync.dma_start(out_ap[b, 0:128, :], ot[:, 0:256])
            nc.sync.dma_start(out_ap[b, 128:256, :], ot[:, 256:512])

    nc.compile()
    return nc


def _get_program():
    if "nc" not in _cache:
        _cache["nc"] = _build_program()
    return _cache["nc"]


def _in_maps(kps):
    rhsA, rhsW = _basis_tables()
    ident = np.eye(128, dtype=np.float16)
    in_maps = []
    for c in range(N_CORES):
        lhsA, lhsW, sig = _core_tables(kps[c * B_LOCAL:(c + 1) * B_LOCAL])
        in_maps.append({
            "lhsA": lhsA, "lhsW": lhsW,
            "rhsA": rhsA, "rhsW": rhsW,
            "sig": sig, "sig2": sig * sig,
            "ident": ident,
        })
    return in_maps


def kernel(kps: np.ndarray) -> np.ndarray:
    kps = np.asarray(kps, np.float32)
    assert kps.shape == (B_TOTAL, E, 2), kps.shape

    nc = _get_program()
    in_maps = _in_maps(kps)

    res = run_bass_kernel_spmd(nc, in_maps, list(range(N_CORES)))
    out = np.concatenate([res.results[c]["out"] for c in range(N_CORES)], axis=0)
    return out.astype(np.float32)


# revision 14
# speedup vs baseline: 1.2538x; 1.1669x over previous
"""Trainium2 Bass kernel for KPToSkl: keypoint-skeleton Gaussian heatmap.

Computes heat[b,y,x] = max_e exp(-gamma * dist^2(pixel, segment_e))
                     = exp(-gamma * min_e dist^2)   (exp is monotone)

Data-parallel over batch: B=32 split as 4 batches per NeuronCore x 8 cores.

Per (b, e) the device computes, over the 256x256 image laid out as
[128 partitions (y mod 128), 512 free (yhalf*256 + x)]:

  PE:  A  = projection coordinate, affine in (y, x), scaled so the segment
            band maps to [-1, 1]  (rank-k matmul vs a shared x-basis)
       W2 = gamma * perpendicular_dist^2, quadratic in (y, x) (rank-k matmul)
  DVE: m  = max(|A|, 1) - 1          (tensor_scalar dual-op: abs_max, subtract)
  ACT: q  = (sigma*m)^2              (Square activation, per-(b,e) scale col)
  DVE: d2 = W2 + q ; M = min(M, d2)
  ACT: heat = exp(-M) at the end of the edge loop.

All per-(b,e) coefficients are host-precomputed into fp16 hi/lo pairs
(products of fp16 values accumulate exactly in fp32 PSUM), so the matmuls
run at the full 1 cycle/row fp16 rate with ~1e-6 accuracy.
"""

from contextlib import ExitStack

import numpy as np

import concourse.bass as bass
import concourse.tile as tile
from concourse import bacc, mybir
from concourse.bass_utils import run_bass_kernel_spmd

N_CORES = 8
B_TOTAL = 32
B_LOCAL = B_TOTAL // N_CORES  # 4
E = 18
H = W = 256
GAMMA = 0.2
SG = float(np.sqrt(GAMMA))
BE = B_LOCAL * E  # 72 (b,e) pairs per core

KA = 6   # lhsT rows for the A matmul
KW = 11  # lhsT rows for the W2 matmul
KP = 2 * (KA + KW)  # packed rows per edge-pair (34)

F16 = mybir.dt.float16
F32 = mybir.dt.float32

_cache = {}


def _split16(v):
    v = np.asarray(v, np.float64)
    hi = v.astype(np.float16)
    lo = (v - hi.astype(np.float64)).astype(np.float16)
    return hi, lo


def _basis_tables():
    """Shared rhs basis matrices (same for every core / batch / edge).

    The x basis is scaled by 2^-8 (exact in fp16) so that the per-edge
    coefficients stay in fp16's *normal* range — subnormal coefficients
    (< 6.1e-5) have an absolute quantization floor of ~6e-8 that, times
    x^2 ~ 65025, would cost ~2e-3 of absolute error.

    Returns the four zero-padded [KP, 512] rhs variants for the packed
    pair lhsT (A/W basis for each half), so that the four matmuls of an
    edge pair share one loaded weight set.
    """
    x = np.arange(W, dtype=np.float64)
    xs = x / 256.0  # exact in fp16 (power-of-2 scale of integers < 2^11)
    ones = np.ones(W, np.float64)
    zero = np.zeros(W, np.float64)
    xs2 = xs * xs
    xs2h, xs2l = _split16(xs2)
    xs2h = xs2h.astype(np.float64)
    xs2l = xs2l.astype(np.float64)

    def row(a, b):
        return np.concatenate([a, b])

    rhsA = np.stack([
        row(xs, xs),      # c1_hi
        row(xs, xs),      # c1_lo
        row(ones, zero),  # d_hi yt0
        row(ones, zero),  # d_lo yt0
        row(zero, ones),  # d_hi yt1
        row(zero, ones),  # d_lo yt1
    ])
    rhsW = np.stack([
        row(xs2h, xs2h),  # C_hi
        row(xs2l, xs2l),  # C_hi
        row(xs2h, xs2h),  # C_lo
        row(xs, zero),    # f_hi yt0
        row(xs, zero),    # f_lo yt0
        row(zero, xs),    # f_hi yt1
        row(zero, xs),    # f_lo yt1
        row(ones, zero),  # g_hi yt0
        row(ones, zero),  # g_lo yt0
        row(zero, ones),  # g_hi yt1
        row(zero, ones),  # g_lo yt1
    ])
    variants = []
    for half, mat in ((0, rhsA), (1, rhsW), (2, rhsA), (3, rhsW)):
        v = np.zeros((KP, 512), np.float64)
        base = (KA + KW) if half >= 2 else 0
        off = base + (KA if half % 2 == 1 else 0)
        v[off:off + mat.shape[0], :] = mat
        variants.append(v.astype(np.float16))
    return variants  # [rA0, rW0, rA1, rW1]


def _core_tables(kps_core):
    """Per-core lhsT coefficient tables + sigma columns.

    kps_core: [B_LOCAL, 18, 2] float (x, y) keypoints in [-1, 1].
    Returns lhsA [KA, BE*128] f16, lhsW [KW, BE*128] f16, sig [128, BE] f32.
    """
    ky = kps_core[:, :, 1].astype(np.float64)
    kx = kps_core[:, :, 0].astype(np.float64)
    PI = np.arange(E)
    PJ = (np.arange(E) + 1) % E
    piy, pix = ky[:, PI], kx[:, PI]
    pjy, pjx = ky[:, PJ], kx[:, PJ]
    vy, vx = piy - pjy, pix - pjx
    vn = np.maximum(vy * vy + vx * vx, 1e-12)
    s = np.sqrt(vn)

    P = 2 * vy / vn
    Q = 2 * vx / vn
    R = -1 - (2 / vn) * (pjy * vy + pjx * vx)
    c1 = 2 * Q * 256.0 / 255.0  # coefficient of xs = x/256

    G = SG * vx / s
    Hc = -SG * vy / s
    J = SG * (pjx * vy - pjy * vx) / s
    c2 = 2 * Hc * 256.0 / 255.0  # coefficient of xs in w
    C = c2 * c2

    p = np.arange(128, dtype=np.float64)
    yc0 = 2 * p / 255.0 - 1
    yc1 = 2 * (p + 128) / 255.0 - 1

    npair = BE // 2
    lhsP = np.zeros((KP, npair, 128), np.float16)
    sig = np.zeros((128, BE), np.float32)

    for b in range(B_LOCAL):
        for e in range(E):
            be = b * E + e
            pi, half = be // 2, be % 2
            base = half * (KA + KW)

            c1h, c1l = _split16(c1[b, e])
            lhsP[base + 0, pi, :] = c1h
            lhsP[base + 1, pi, :] = c1l
            for yt, yc in ((0, yc0), (1, yc1)):
                d = P[b, e] * yc + (R[b, e] - Q[b, e])
                dh, dl = _split16(d)
                lhsP[base + 2 + 2 * yt, pi, :] = dh
                lhsP[base + 3 + 2 * yt, pi, :] = dl

            wb = base + KA
            Ch, Cl = _split16(C[b, e])
            lhsP[wb + 0, pi, :] = Ch
            lhsP[wb + 1, pi, :] = Ch
            lhsP[wb + 2, pi, :] = Cl
            for yt, yc in ((0, yc0), (1, yc1)):
                ecol = G[b, e] * yc + (J[b, e] - Hc[b, e])
                f = 2 * c2[b, e] * ecol
                g = ecol * ecol
                fh, fl = _split16(f)
                gh, gl = _split16(g)
                lhsP[wb + 3 + 2 * yt, pi, :] = fh
                lhsP[wb + 4 + 2 * yt, pi, :] = fl
                lhsP[wb + 7 + 2 * yt, pi, :] = gh
                lhsP[wb + 8 + 2 * yt, pi, :] = gl

            sig[:, be] = SG * s[b, e] / 2

    return lhsP.reshape(KP, npair * 128), sig


DVE_Q_MOD = 5  # every DVE_Q_MOD-th edge computes its square on DVE (phi split)
NPAIR = BE // 2
ZDT = F16  # dtype of the |A| / m middle chain


def _build_program():
    nc = bacc.Bacc(
        "TRN2",
        target_bir_lowering=False,
        debug=False,
        num_devices=N_CORES,
    )

    lhsP_d = nc.declare_dram_parameter("lhsP", [KP, NPAIR * 128], F16, isOutput=False)
    rA0_d = nc.declare_dram_parameter("rA0", [KP, 512], F16, isOutput=False)
    rW0_d = nc.declare_dram_parameter("rW0", [KP, 512], F16, isOutput=False)
    rA1_d = nc.declare_dram_parameter("rA1", [KP, 512], F16, isOutput=False)
    rW1_d = nc.declare_dram_parameter("rW1", [KP, 512], F16, isOutput=False)
    sig_d = nc.declare_dram_parameter("sig", [128, BE], F32, isOutput=False)
    sig2_d = nc.declare_dram_parameter("sig2", [128, BE], F32, isOutput=False)
    out_d = nc.declare_dram_parameter("out", [B_LOCAL, H, W], F32, isOutput=True)

    with tile.TileContext(nc) as tc, ExitStack() as ctx:
        const = ctx.enter_context(tc.tile_pool(name="const", bufs=1))
        psum = ctx.enter_context(tc.tile_pool(name="psum", bufs=1, space="PSUM"))
        work = ctx.enter_context(tc.tile_pool(name="work", bufs=3))
        mpool = ctx.enter_context(tc.tile_pool(name="mins", bufs=4))
        opool = ctx.enter_context(tc.tile_pool(name="outs", bufs=2))

        lhsP_t = const.tile([KP, NPAIR * 128], F16)
        nc.sync.dma_start(lhsP_t[:], lhsP_d.ap())
        rA0_t = const.tile([KP, 512], F16)
        nc.sync.dma_start(rA0_t[:], rA0_d.ap())
        rW0_t = const.tile([KP, 512], F16)
        nc.sync.dma_start(rW0_t[:], rW0_d.ap())
        rA1_t = const.tile([KP, 512], F16)
        nc.sync.dma_start(rA1_t[:], rA1_d.ap())
        rW1_t = const.tile([KP, 512], F16)
        nc.sync.dma_start(rW1_t[:], rW1_d.ap())
        sig_t = const.tile([128, BE], F32)
        nc.sync.dma_start(sig_t[:], sig_d.ap())
        sig2_t = const.tile([128, BE], F32)
        nc.sync.dma_start(sig2_t[:], sig2_d.ap())
        zcol = const.tile([128, 1], F32)
        nc.gpsimd.memset(zcol[:], 0.0)
        rz_t = const.tile([KP, 512], F16)
        nc.gpsimd.memset(rz_t[:], 0.0)

        # Persistent PSUM slots: 2 x [128,1024] for A-pairs, 4 x [128,512]
        # for d2 banks. The 4 one-time "dummy" matmuls below set the
        # has_written bits of the d2 banks; afterwards every pair's q is
        # ACT-written into the bank (plain write, bits survive) and the W2
        # matmul with start=False accumulates on top: d2 = q + W2.
        aas = [psum.tile([128, 1024], F32, name=f"aa{i}") for i in range(2)]
        pds = [psum.tile([128, 512], F32, name=f"pd{i}") for i in range(4)]
        for j in range(4):
            nc.tensor.matmul(
                pds[j][:], lhsP_t[:, 0:128], rz_t[:],
                start=True, stop=True, skip_group_check=True,
            )

        out_ap = out_d.ap()

        rAs, rWs = (rA0_t, rA1_t), (rW0_t, rW1_t)
        pcount = 0
        dcount = 0
        for b in range(B_LOCAL):
            # Ping-pong min accumulators (avoids in-place DVE read/write).
            m0 = mpool.tile([128, 512], F32, tag="m0")
            m1 = mpool.tile([128, 512], F32, tag="m1")
            nc.gpsimd.memset(m0[:], 30.0)
            cur, nxt = m0, m1
            for ep in range(E // 2):
                pi = b * (E // 2) + ep
                lhs = lhsP_t[:, pi * 128:(pi + 1) * 128]
                aa = aas[pcount % 2]
                pcount += 1
                for half in (0, 1):
                    nc.tensor.matmul(
                        aa[:, half * 512:(half + 1) * 512], lhs, rAs[half][:],
                        start=True, stop=True, skip_group_check=True,
                    )
                zt = work.tile([128, 1024], ZDT, tag="zt")
                nc.scalar.activation(
                    zt[:], aa[:], mybir.ActivationFunctionType.Abs,
                    bias=zcol[:], scale=1.0,
                )
                mt = work.tile([128, 1024], ZDT, tag="mt")
                nc.vector.tensor_scalar(
                    mt[:], zt[:], 1.0, 1.0,
                    mybir.AluOpType.max, mybir.AluOpType.subtract,
                )
                for half in (0, 1):
                    be = 2 * pi + half
                    mh = mt[:, half * 512:(half + 1) * 512]
                    pd = pds[dcount % 4]
                    dcount += 1
                    if (be % DVE_Q_MOD) == DVE_Q_MOD - 1:
                        # sigma^2 * m * m on DVE, written into the d2 bank
                        nc.vector.scalar_tensor_tensor(
                            pd[:], mh, sig2_t[:, be:be + 1], mh,
                            mybir.AluOpType.mult, mybir.AluOpType.mult,
                        )
                    else:
                        nc.scalar.activation(
                            pd[:], mh, mybir.ActivationFunctionType.Square,
                            bias=zcol[:], scale=sig_t[:, be:be + 1],
                        )
                    nc.tensor.matmul(
                        pd[:], lhs, rWs[half][:],
                        start=False, stop=True, skip_group_check=True,
                    )
                    nc.vector.tensor_tensor(
                        nxt[:], cur[:], pd[:], mybir.AluOpType.min
                    )
                    cur, nxt = nxt, cur
            ot = opool.tile([128, 512], F32)
            nc.scalar.activation(
                ot[:], cur[:], mybir.ActivationFunctionType.Exp,
                bias=zcol[:], scale=-1.0,
            )
            nc.sync.dma_start(out_ap[b, 0:128, :], ot[:, 0:256])
            nc.sync.dma_start(out_ap[b, 128:256, :], ot[:, 256:512])

    nc.compile()
    return nc


def _get_program():
    if "nc" not in _cache:
        _cache["nc"] = _build_program()
    return _cache["nc"]


def _in_maps(kps):
    rA0, rW0, rA1, rW1 = _basis_tables()
    in_maps = []
    for c in range(N_CORES):
        lhsP, sig = _core_tables(kps[c * B_LOCAL:(c + 1) * B_LOCAL])
        in_maps.append({
            "lhsP": lhsP,
            "rA0": rA0, "rW0": rW0, "rA1": rA1, "rW1": rW1,
            "sig": sig, "sig2": sig * sig,
        })
    return in_maps


def kernel(kps: np.ndarray) -> np.ndarray:
    kps = np.asarray(kps, np.float32)
    assert kps.shape == (B_TOTAL, E, 2), kps.shape

    nc = _get_program()
    in_maps = _in_maps(kps)

    res = run_bass_kernel_spmd(nc, in_maps, list(range(N_CORES)))
    out = np.concatenate([res.results[c]["out"] for c in range(N_CORES)], axis=0)
    return out.astype(np.float32)
